# revision 17
# baseline (speedup 1.0000x reference)
"""CapInfoNCE loss kernel for Trainium2 (8 NeuronCores, SPMD).

Sharding: the Bo (o/u batch) axis is split across the 8 cores (12 columns
each). Each core holds the full w (captions) and computes, for its 12
o-columns:
  KuT  [Dk, 1200]  = Wku^T u_s^T + bku      (dk on partitions, bf16)
  VoT  [Dv, 1200]  = Wfo^T o_s^T + bfo
  KwT  [Dk, 2880]  = Wkw^T w^T + bkw
  VwT  [Dv, 2880]  = Wfw^T w^T + bfw
  Vo   [100,12,Dv] = transpose(VoT) per column (token on partitions)
then per group of 4 captions (120 = 4x30 rows on partitions):
  S      = KwT_g^T KuT / 16 -> exp (no max-sub; |S| <~ 5) = attE  (f32)
  sums   = segment-sum over To; recip = 1/sums
  att    = attE * recip                     -> DRAM [w,t,b,o] f32
  G      = VwT_g^T VoT  (PSUM)
  logits = segsum(attE * G) * recip         -> DRAM [w,t,b]
  per b: attT = transpose(attE[:,b,:]); attV = (attT^T Vo_b)*recip -> DRAM
The tiny contrastive tail (log_softmax over Bo + mask + mean) runs on the
host over the gathered [96,96,30] logits.

Matmul operands are bf16 (fp32 PSUM accumulation). The activation/weight
transposed loads use the XBAR DMA-transpose on host-prepared, row-padded
bf16 copies of the inputs; everything downstream of PSUM stays fp32.
"""

import sys

for _p in ("/opt/trn_rl_repo",):
    if _p not in sys.path:
        sys.path.insert(0, _p)

import ml_dtypes
import numpy as np

BO, TO, DO = 96, 100, 1024
BW, TW, DW = 96, 30, 768
DU, DK, DV = 2048, 256, 256
NCORES = 8
BL = BO // NCORES          # 12 o-columns per core
TOK = BL * TO              # 1200 local o/u tokens
TOKP = 1280                # padded to a multiple of 128 for DMA-transpose
WTOK = BW * TW             # 2880 caption tokens
WTOKP = 3072
WTOKA = 2944               # KwT/VwT col allocation (2880 + 64 zero tail)
WG = 24                    # caption groups of 4 (120 partition rows)


def _emit(ctx, tc, outs, ins):
    import concourse.mybir as mybir
    from concourse.bass import ts
    from concourse.masks import make_identity

    nc = tc.nc
    f32 = mybir.dt.float32
    bf16 = mybir.dt.bfloat16
    AX = mybir.AxisListType.X
    OP = mybir.AluOpType
    AFT = mybir.ActivationFunctionType

    u_d, o_d, w_d = ins["u16T"], ins["o16T"], ins["w16T"]
    att_d, attV_d, lg_d = outs["att"], outs["attV"], outs["lg"]

    def mm(out, lhsT, rhs, start, stop):
        assert lhsT.dtype == bf16 and rhs.dtype == bf16
        nc.tensor.matmul(out, lhsT, rhs, start=start, stop=stop)

    _alt = [0]

    def copy_alt(out, in_):
        # Balance PSUM->SBUF copies between DVE and ACT.
        _alt[0] ^= 1
        if _alt[0]:
            nc.vector.tensor_copy(out, in_)
        else:
            nc.scalar.copy(out, in_)

    cpool = ctx.enter_context(tc.tile_pool(name="consts", bufs=1))
    bigpool = ctx.enter_context(tc.tile_pool(name="resident", bufs=1))
    apool = ctx.enter_context(tc.tile_pool(name="attwork", bufs=2))
    spool = ctx.enter_context(tc.tile_pool(name="small", bufs=2))
    outpool = ctx.enter_context(tc.tile_pool(name="outs", bufs=3))

    ptrf = ctx.enter_context(tc.tile_pool(name="ptrf", bufs=2, space="PSUM"))
    pmm = ctx.enter_context(tc.tile_pool(name="pmm", bufs=2, space="PSUM"))
    pS = ctx.enter_context(tc.tile_pool(name="pS", bufs=2, space="PSUM"))
    pG = ctx.enter_context(tc.tile_pool(name="pG", bufs=2, space="PSUM"))

    # ---- constants ----------------------------------------------------
    Wku_sb = cpool.tile([128, DU // 128, DK], bf16, tag="Wku")
    nc.sync.dma_start(Wku_sb, ins["Wku16"].rearrange("(a p) n -> p a n", p=128))
    Wkw_sb = cpool.tile([128, DW // 128, DK], bf16, tag="Wkw")
    nc.sync.dma_start(Wkw_sb, ins["Wkw16"].rearrange("(a p) n -> p a n", p=128))
    Wfo_sb = cpool.tile([128, DO // 128, DV], bf16, tag="Wfo")
    nc.sync.dma_start(Wfo_sb, ins["Wfo16"].rearrange("(a p) n -> p a n", p=128))
    Wfw_sb = cpool.tile([128, DW // 128, DV], bf16, tag="Wfw")
    nc.sync.dma_start(Wfw_sb, ins["Wfw16"].rearrange("(a p) n -> p a n", p=128))

    bku_sb = cpool.tile([128, 2], f32, tag="bku")
    nc.sync.dma_start(bku_sb, ins["bku"].rearrange("(a p) -> p a", p=128))
    bkw_sb = cpool.tile([128, 2], f32, tag="bkw")
    nc.sync.dma_start(bkw_sb, ins["bkw"].rearrange("(a p) -> p a", p=128))
    bfo_sb = cpool.tile([128, 2], f32, tag="bfo")
    nc.sync.dma_start(bfo_sb, ins["bfo"].rearrange("(a p) -> p a", p=128))
    bfw_sb = cpool.tile([128, 2], f32, tag="bfw")
    nc.sync.dma_start(bfw_sb, ins["bfw"].rearrange("(a p) -> p a", p=128))

    ident = cpool.tile([128, 128], f32, tag="ident")
    make_identity(nc, ident)
    ident16 = cpool.tile([128, 128], bf16, tag="ident16")
    make_identity(nc, ident16)

    # ---- resident operand tensors (bf16) ------------------------------
    uT = bigpool.tile([128, DU // 128, TOK], bf16, tag="uT")
    oT = bigpool.tile([128, DO // 128, TOK], bf16, tag="oT")
    wT = bigpool.tile([128, DW // 128, WTOK], bf16, tag="wT")
    KuT = bigpool.tile([128, 2, TOK], bf16, tag="KuT")
    VoT = bigpool.tile([128, 2, TOK], bf16, tag="VoT")
    Vo = bigpool.tile([128, BL, DV], bf16, tag="Vo")
    KwT = bigpool.tile([128, 2, WTOKA], bf16, tag="KwT")
    VwT = bigpool.tile([128, 2, WTOKA], bf16, tag="VwT")
    # zero tails so padded stationary reads (and last-group rows) are clean
    nc.vector.memset(KwT[:, :, WTOK:], 0.0)
    nc.vector.memset(VwT[:, :, WTOK:], 0.0)

    # ---- transposed operand loads (host supplies X^T), both queues -----
    _dq = [0]

    def dma_q(out, in_):
        _dq[0] ^= 1
        (nc.sync if _dq[0] else nc.scalar).dma_start(out, in_)

    for k in range(DW // 128):
        dma_q(wT[:, k, :], w_d[ts(k, 128)])
    for k in range(DO // 128):
        dma_q(oT[:, k, :], o_d[ts(k, 128)])
    for k in range(DU // 128):
        dma_q(uT[:, k, :], u_d[ts(k, 128)])

    # ---- projections ---------------------------------------------------
    for g in range(6):
        for c in range(2):
            pm = pmm.tile([128, 480], f32, tag="pmm")
            for k in range(DW // 128):
                mm(pm, Wkw_sb[:, k, ts(c, 128)], wT[:, k, ts(g, 480)],
                   start=(k == 0), stop=(k == DW // 128 - 1))
            nc.vector.tensor_scalar_add(KwT[:, c, ts(g, 480)], pm,
                                        bkw_sb[:, c:c + 1])
            pm = pmm.tile([128, 480], f32, tag="pmm")
            for k in range(DW // 128):
                mm(pm, Wfw_sb[:, k, ts(c, 128)], wT[:, k, ts(g, 480)],
                   start=(k == 0), stop=(k == DW // 128 - 1))
            nc.vector.tensor_scalar_add(VwT[:, c, ts(g, 480)], pm,
                                        bfw_sb[:, c:c + 1])
    for q in range(3):
        for c in range(2):
            pm = pmm.tile([128, 480], f32, tag="pmm")
            for k in range(DU // 128):
                mm(pm[:, :400], Wku_sb[:, k, ts(c, 128)], uT[:, k, ts(q, 400)],
                   start=(k == 0), stop=(k == DU // 128 - 1))
            nc.vector.tensor_scalar_add(KuT[:, c, ts(q, 400)], pm[:, :400],
                                        bku_sb[:, c:c + 1])
        for c in range(2):
            pm = pmm.tile([128, 480], f32, tag="pmm")
            for k in range(DO // 128):
                mm(pm[:, :400], Wfo_sb[:, k, ts(c, 128)], oT[:, k, ts(q, 400)],
                   start=(k == 0), stop=(k == DO // 128 - 1))
            nc.vector.tensor_scalar_add(VoT[:, c, ts(q, 400)], pm[:, :400],
                                        bfo_sb[:, c:c + 1])

    # ---- Vo natural layout via PE transpose of VoT ---------------------
    for b in range(BL):
        for c in range(2):
            ptb = pS.tile([128, 128], bf16, tag="pS")
            nc.tensor.transpose(ptb[:TO, :], VoT[:, c, b * TO:(b + 1) * TO],
                                ident16)
            copy_alt(Vo[:TO, b, ts(c, 128)], ptb[:TO, :])

    # ---- phase 2: attention / outputs per caption-group ----------------
    for wg in range(WG):
        r0 = wg * 120
        attE = apool.tile([128, BL, TO], f32, tag="attE")
        for ch in range(3):
            ps = pS.tile([128, 400], f32, tag="pS")
            mm(ps, KwT[:, 0, r0:r0 + 128], KuT[:, 0, ts(ch, 400)],
               start=True, stop=False)
            mm(ps, KwT[:, 1, r0:r0 + 128], KuT[:, 1, ts(ch, 400)],
               start=False, stop=True)
            nc.scalar.activation(
                attE[:120, ch * 4:(ch + 1) * 4, :],
                ps[:120].rearrange("p (b t) -> p b t", t=TO),
                AFT.Exp, scale=1.0 / 16.0)

        sums = spool.tile([128, BL], f32, tag="sums")
        nc.vector.tensor_reduce(sums[:120], attE[:120], axis=AX, op=OP.add)
        recip = spool.tile([128, BL], f32, tag="recip")
        nc.vector.reciprocal(recip[:120], sums[:120])

        attO = apool.tile([128, BL, TO], f32, tag="attO")
        nc.gpsimd.tensor_tensor(
            attO[:120], attE[:120],
            recip[:120, :, None].to_broadcast((120, BL, TO)), OP.mult)
        dma_q(att_d[ts(wg, 4)].rearrange("w t b o -> (w t) b o"), attO[:120])

        lacc = spool.tile([128, BL], f32, tag="lacc")
        for ch in range(3):
            pg = pG.tile([128, 400], f32, tag="pG")
            mm(pg, VwT[:, 0, r0:r0 + 128], VoT[:, 0, ts(ch, 400)],
               start=True, stop=False)
            mm(pg, VwT[:, 1, r0:r0 + 128], VoT[:, 1, ts(ch, 400)],
               start=False, stop=True)
            scr = spool.tile([128, 4, TO], f32, tag="scr")
            nc.vector.tensor_tensor(
                scr[:120], attE[:120, ch * 4:(ch + 1) * 4, :],
                pg[:120].rearrange("p (b t) -> p b t", t=TO), OP.mult)
            nc.vector.tensor_reduce(
                lacc[:120, ch * 4:(ch + 1) * 4], scr[:120], axis=AX,
                op=OP.add)
        lout = spool.tile([128, BL], f32, tag="lout")
        nc.vector.tensor_mul(lout[:120], lacc[:120], recip[:120])
        dma_q(lg_d[ts(wg, 4)].rearrange("w t b -> (w t) b"), lout[:120])

        for h in range(2):
            av6 = outpool.tile([128, 6, DV], f32, tag="attV6")
            for pr in range(3):
                pa = pmm.tile([128, 512], f32, tag="pmm")
                for j2 in range(2):
                    j = pr * 2 + j2
                    b = h * 6 + j
                    pt = ptrf.tile([128, 128], f32, tag="ptrf")
                    nc.tensor.transpose(pt[:TO, :120], attE[:120, b, :],
                                        ident[:120, :120])
                    aT = outpool.tile([128, 120], bf16, tag="attT")
                    if b % 3 < 2:
                        nc.vector.tensor_copy(aT[:TO], pt[:TO, :120])
                    else:
                        nc.scalar.copy(aT[:TO], pt[:TO, :120])
                    mm(pa[:120, j2 * DV:(j2 + 1) * DV], aT[:TO, :],
                       Vo[:TO, b, :], start=(j2 == 0), stop=(j2 == 1))
                j = pr * 2
                b = h * 6 + j
                if pr % 2 == 0:
                    nc.vector.tensor_tensor(
                        av6[:120, j:j + 2, :],
                        pa[:120].rearrange("p (b d) -> p b d", d=DV),
                        recip[:120, b:b + 2, None].to_broadcast((120, 2, DV)),
                        OP.mult)
                else:
                    nc.scalar.activation(av6[:120, j, :], pa[:120, :DV],
                                         AFT.Copy, scale=recip[:120, b:b + 1])
                    nc.scalar.activation(av6[:120, j + 1, :],
                                         pa[:120, DV:2 * DV],
                                         AFT.Copy,
                                         scale=recip[:120, b + 1:b + 2])
            dma_q(attV_d[ts(wg, 4), :, h * 6:(h + 1) * 6]
                  .rearrange("w t b d -> (w t) b d"), av6[:120])


def _build():
    from contextlib import ExitStack

    import concourse.mybir as mybir
    import concourse.tile as tile
    from concourse import bacc

    f32 = mybir.dt.float32
    bf16 = mybir.dt.bfloat16
    nc = bacc.Bacc("TRN2", target_bir_lowering=False, debug=False,
                   num_devices=NCORES)

    def di(name, shape, dt):
        return nc.dram_tensor(name, shape, dt, kind="ExternalInput").ap()

    ins = {
        "u16T": di("u16T", [DU, TOK], bf16),
        "o16T": di("o16T", [DO, TOK], bf16),
        "w16T": di("w16T", [DW, WTOK], bf16),
        "Wku16": di("Wku16", [DU, DK], bf16),
        "Wkw16": di("Wkw16", [DW, DK], bf16),
        "Wfo16": di("Wfo16", [DO, DV], bf16),
        "Wfw16": di("Wfw16", [DW, DV], bf16),
        "bku": di("bku", [DK], f32),
        "bkw": di("bkw", [DK], f32),
        "bfo": di("bfo", [DV], f32),
        "bfw": di("bfw", [DV], f32),
    }
    outs = {
        "att": nc.dram_tensor("att", [BW, TW, BL, TO], f32,
                              kind="ExternalOutput").ap(),
        "attV": nc.dram_tensor("attV", [BW, TW, BL, DV], f32,
                               kind="ExternalOutput").ap(),
        "lg": nc.dram_tensor("lg", [BW, TW, BL], f32,
                             kind="ExternalOutput").ap(),
    }

    with tile.TileContext(nc) as tc:
        with ExitStack() as ctx:
            _emit(ctx, tc, outs, ins)
    nc.compile()
    return nc


_NC_CACHE = []
LAST_EXEC_NS = [None]


def _get_nc():
    if not _NC_CACHE:
        _NC_CACHE.append(_build())
    return _NC_CACHE[0]


def _pad_rows(a, rows):
    out = np.zeros((rows,) + a.shape[1:], dtype=a.dtype)
    out[: a.shape[0]] = a
    return out


def kernel(**inputs):
    import os

    from concourse.bass_utils import run_bass_kernel_spmd

    nc = _get_nc()
    bf = ml_dtypes.bfloat16

    u = np.asarray(inputs["u"], np.float32)
    o = np.asarray(inputs["o"], np.float32)
    w = np.asarray(inputs["w"], np.float32)
    mask = np.asarray(inputs["mask"], np.float32)

    shared = {
        "w16T": np.ascontiguousarray(w.reshape(WTOK, DW).T.astype(bf)),
        "Wku16": np.asarray(inputs["Wku"], np.float32).astype(bf),
        "Wkw16": np.asarray(inputs["Wkw"], np.float32).astype(bf),
        "Wfo16": np.asarray(inputs["Wfo"], np.float32).astype(bf),
        "Wfw16": np.asarray(inputs["Wfw"], np.float32).astype(bf),
        "bku": np.ascontiguousarray(np.asarray(inputs["bku"], np.float32)),
        "bkw": np.ascontiguousarray(np.asarray(inputs["bkw"], np.float32)),
        "bfo": np.ascontiguousarray(np.asarray(inputs["bfo"], np.float32)),
        "bfw": np.ascontiguousarray(np.asarray(inputs["bfw"], np.float32)),
    }

    in_maps = []
    for k in range(NCORES):
        m = dict(shared)
        m["u16T"] = np.ascontiguousarray(
            u[k * BL:(k + 1) * BL].reshape(TOK, DU).T.astype(bf))
        m["o16T"] = np.ascontiguousarray(
            o[k * BL:(k + 1) * BL].reshape(TOK, DO).T.astype(bf))
        in_maps.append(m)

    trace = bool(int(os.environ.get("KERNEL_TRACE", "0")))
    res = run_bass_kernel_spmd(nc, in_maps, core_ids=list(range(NCORES)),
                               trace=trace)
    results = res.results
    LAST_EXEC_NS[0] = res.exec_time_ns
    if res.instructions_and_trace is not None:
        print("trace:", res.instructions_and_trace[1])

    att = np.empty((BW, BO, TW, TO), np.float32)
    attV = np.empty((BW, BO, TW, DV), np.float32)
    logits = np.empty((BW, BO, TW), np.float32)
    for k in range(NCORES):
        bs = slice(k * BL, (k + 1) * BL)
        att[:, bs] = results[k]["att"].transpose(0, 2, 1, 3)
        attV[:, bs] = results[k]["attV"].transpose(0, 2, 1, 3)
        logits[:, bs] = results[k]["lg"].transpose(0, 2, 1)

    # contrastive tail on host: log_softmax over Bo, mask, mean of diagonal
    mx = logits.max(axis=1, keepdims=True)
    lse = np.log(np.exp(logits - mx).sum(axis=1, keepdims=True)) + mx
    lsm = logits - lse
    m = mask[:, None, :]
    lsm = (1.0 - m) * lsm
    nn = (1.0 - m).sum(axis=2, keepdims=True)
    lsm = lsm / (nn + 1e-6)
    loss = -np.mean(np.diagonal(lsm.sum(axis=2)))

    return np.float32(loss), att, attV


# revision 19
# speedup vs baseline: 1.2796x; 1.2796x over previous
"""CapInfoNCE loss kernel for Trainium2 (8 NeuronCores, SPMD).

Sharding: the Bo (o/u batch) axis is split across the 8 cores (12 columns
each). Each core holds the full w (captions) and computes, for its 12
o-columns:
  KuT  [Dk, 1200]  = Wku^T u_s^T + bku      (dk on partitions, bf16)
  VoT  [Dv, 1200]  = Wfo^T o_s^T + bfo
  KwT  [Dk, 2880]  = Wkw^T w^T + bkw
  VwT  [Dv, 2880]  = Wfw^T w^T + bfw
  Vo   [100,12,Dv] = transpose(VoT) per column (token on partitions)
then per group of 4 captions (120 = 4x30 rows on partitions):
  S      = KwT_g^T KuT / 16 -> exp (no max-sub; |S| <~ 5) = attE  (f32)
  sums   = segment-sum over To; recip = 1/sums
  att    = attE * recip                     -> DRAM [w,t,b,o] f32
  G      = VwT_g^T VoT  (PSUM)
  logits = segsum(attE * G) * recip         -> DRAM [w,t,b]
  per b: attT = transpose(attE[:,b,:]); attV = (attT^T Vo_b)*recip -> DRAM
The tiny contrastive tail (log_softmax over Bo + mask + mean) runs on the
host over the gathered [96,96,30] logits.

Matmul operands are bf16 (fp32 PSUM accumulation). The activation/weight
transposed loads use the XBAR DMA-transpose on host-prepared, row-padded
bf16 copies of the inputs; everything downstream of PSUM stays fp32.
"""

import sys

for _p in ("/opt/trn_rl_repo",):
    if _p not in sys.path:
        sys.path.insert(0, _p)

import ml_dtypes
import numpy as np

BO, TO, DO = 96, 100, 1024
BW, TW, DW = 96, 30, 768
DU, DK, DV = 2048, 256, 256
NCORES = 8
BL = BO // NCORES          # 12 o-columns per core
TOK = BL * TO              # 1200 local o/u tokens
TOKP = 1280                # padded to a multiple of 128 for DMA-transpose
WTOK = BW * TW             # 2880 caption tokens
WTOKP = 3072
WTOKA = 2944               # KwT/VwT col allocation (2880 + 64 zero tail)
WG = 24                    # caption groups of 4 (120 partition rows)


def _emit(ctx, tc, outs, ins):
    import concourse.mybir as mybir
    from concourse.bass import ts
    from concourse.masks import make_identity

    nc = tc.nc
    f32 = mybir.dt.float32
    bf16 = mybir.dt.bfloat16
    AX = mybir.AxisListType.X
    OP = mybir.AluOpType
    AFT = mybir.ActivationFunctionType

    u_d, o_d, w_d = ins["u16T"], ins["o16T"], ins["w16T"]
    att_d, attV_d, lg_d = outs["att"], outs["attV"], outs["lg"]

    def mm(out, lhsT, rhs, start, stop):
        assert lhsT.dtype == bf16 and rhs.dtype == bf16
        nc.tensor.matmul(out, lhsT, rhs, start=start, stop=stop)

    _alt = [0]

    def copy_alt(out, in_):
        # Balance PSUM->SBUF copies between DVE and ACT.
        _alt[0] ^= 1
        if _alt[0]:
            nc.vector.tensor_copy(out, in_)
        else:
            nc.scalar.copy(out, in_)

    cpool = ctx.enter_context(tc.tile_pool(name="consts", bufs=1))
    bigpool = ctx.enter_context(tc.tile_pool(name="resident", bufs=1))
    apool = ctx.enter_context(tc.tile_pool(name="attwork", bufs=2))
    spool = ctx.enter_context(tc.tile_pool(name="small", bufs=2))
    outpool = ctx.enter_context(tc.tile_pool(name="outs", bufs=3))

    ptrf = ctx.enter_context(tc.tile_pool(name="ptrf", bufs=2, space="PSUM"))
    pmm = ctx.enter_context(tc.tile_pool(name="pmm", bufs=2, space="PSUM"))
    pS = ctx.enter_context(tc.tile_pool(name="pS", bufs=2, space="PSUM"))
    pG = ctx.enter_context(tc.tile_pool(name="pG", bufs=2, space="PSUM"))

    # ---- constants ----------------------------------------------------
    Wku_sb = cpool.tile([128, DU // 128, DK], bf16, tag="Wku")
    nc.sync.dma_start(Wku_sb, ins["Wku16"].rearrange("(a p) n -> p a n", p=128))
    Wkw_sb = cpool.tile([128, DW // 128, DK], bf16, tag="Wkw")
    nc.sync.dma_start(Wkw_sb, ins["Wkw16"].rearrange("(a p) n -> p a n", p=128))
    Wfo_sb = cpool.tile([128, DO // 128, DV], bf16, tag="Wfo")
    nc.sync.dma_start(Wfo_sb, ins["Wfo16"].rearrange("(a p) n -> p a n", p=128))
    Wfw_sb = cpool.tile([128, DW // 128, DV], bf16, tag="Wfw")
    nc.sync.dma_start(Wfw_sb, ins["Wfw16"].rearrange("(a p) n -> p a n", p=128))

    bku_sb = cpool.tile([128, 2], f32, tag="bku")
    nc.sync.dma_start(bku_sb, ins["bku"].rearrange("(a p) -> p a", p=128))
    bkw_sb = cpool.tile([128, 2], f32, tag="bkw")
    nc.sync.dma_start(bkw_sb, ins["bkw"].rearrange("(a p) -> p a", p=128))
    bfo_sb = cpool.tile([128, 2], f32, tag="bfo")
    nc.sync.dma_start(bfo_sb, ins["bfo"].rearrange("(a p) -> p a", p=128))
    bfw_sb = cpool.tile([128, 2], f32, tag="bfw")
    nc.sync.dma_start(bfw_sb, ins["bfw"].rearrange("(a p) -> p a", p=128))

    ident = cpool.tile([128, 128], f32, tag="ident")
    make_identity(nc, ident)
    ident16 = cpool.tile([128, 128], bf16, tag="ident16")
    make_identity(nc, ident16)

    # ---- resident operand tensors (bf16) ------------------------------
    uT = bigpool.tile([128, DU // 128, TOK], bf16, tag="uT")
    oT = bigpool.tile([128, DO // 128, TOK], bf16, tag="oT")
    wT = bigpool.tile([128, DW // 128, WTOK], bf16, tag="wT")
    KuT = bigpool.tile([128, 2, TOK], bf16, tag="KuT")
    VoT = bigpool.tile([128, 2, TOK], bf16, tag="VoT")
    Vo = bigpool.tile([128, BL, DV], bf16, tag="Vo")
    KwT = bigpool.tile([128, 2, WTOKA], bf16, tag="KwT")
    VwT = bigpool.tile([128, 2, WTOKA], bf16, tag="VwT")
    # zero tails so padded stationary reads (and last-group rows) are clean
    nc.vector.memset(KwT[:, :, WTOK:], 0.0)
    nc.vector.memset(VwT[:, :, WTOK:], 0.0)

    # ---- transposed operand loads (host supplies X^T), both queues -----
    _dq = [0]

    def dma_q(out, in_):
        _dq[0] ^= 1
        (nc.sync if _dq[0] else nc.scalar).dma_start(out, in_)

    for k in range(DW // 128):
        dma_q(wT[:, k, :], w_d[ts(k, 128)])
    for k in range(DO // 128):
        dma_q(oT[:, k, :], o_d[ts(k, 128)])
    for k in range(DU // 128):
        dma_q(uT[:, k, :], u_d[ts(k, 128)])

    # ---- projections ---------------------------------------------------
    for g in range(6):
        for c in range(2):
            pm = pmm.tile([128, 480], f32, tag="pmm")
            for k in range(DW // 128):
                mm(pm, Wkw_sb[:, k, ts(c, 128)], wT[:, k, ts(g, 480)],
                   start=(k == 0), stop=(k == DW // 128 - 1))
            nc.vector.tensor_scalar_add(KwT[:, c, ts(g, 480)], pm,
                                        bkw_sb[:, c:c + 1])
            pm = pmm.tile([128, 480], f32, tag="pmm")
            for k in range(DW // 128):
                mm(pm, Wfw_sb[:, k, ts(c, 128)], wT[:, k, ts(g, 480)],
                   start=(k == 0), stop=(k == DW // 128 - 1))
            nc.vector.tensor_scalar_add(VwT[:, c, ts(g, 480)], pm,
                                        bfw_sb[:, c:c + 1])
    for q in range(3):
        for c in range(2):
            pm = pmm.tile([128, 480], f32, tag="pmm")
            for k in range(DU // 128):
                mm(pm[:, :400], Wku_sb[:, k, ts(c, 128)], uT[:, k, ts(q, 400)],
                   start=(k == 0), stop=(k == DU // 128 - 1))
            nc.vector.tensor_scalar_add(KuT[:, c, ts(q, 400)], pm[:, :400],
                                        bku_sb[:, c:c + 1])
        for c in range(2):
            pm = pmm.tile([128, 480], f32, tag="pmm")
            for k in range(DO // 128):
                mm(pm[:, :400], Wfo_sb[:, k, ts(c, 128)], oT[:, k, ts(q, 400)],
                   start=(k == 0), stop=(k == DO // 128 - 1))
            nc.vector.tensor_scalar_add(VoT[:, c, ts(q, 400)], pm[:, :400],
                                        bfo_sb[:, c:c + 1])

    # ---- Vo natural layout via PE transpose of VoT ---------------------
    for b in range(BL):
        for c in range(2):
            ptb = pS.tile([128, 128], bf16, tag="pS")
            nc.tensor.transpose(ptb[:TO, :], VoT[:, c, b * TO:(b + 1) * TO],
                                ident16)
            copy_alt(Vo[:TO, b, ts(c, 128)], ptb[:TO, :])

    # ---- phase 2: attention / outputs per caption-group ----------------
    for wg in range(WG):
        r0 = wg * 120
        attE = apool.tile([128, BL, TO], f32, tag="attE")
        for ch in range(3):
            ps = pS.tile([128, 400], f32, tag="pS")
            mm(ps, KwT[:, 0, r0:r0 + 128], KuT[:, 0, ts(ch, 400)],
               start=True, stop=False)
            mm(ps, KwT[:, 1, r0:r0 + 128], KuT[:, 1, ts(ch, 400)],
               start=False, stop=True)
            nc.scalar.activation(
                attE[:120, ch * 4:(ch + 1) * 4, :],
                ps[:120].rearrange("p (b t) -> p b t", t=TO),
                AFT.Exp, scale=1.0 / 16.0)

        sums = spool.tile([128, BL], f32, tag="sums")
        nc.vector.tensor_reduce(sums[:120], attE[:120], axis=AX, op=OP.add)
        recip = spool.tile([128, BL], f32, tag="recip")
        nc.vector.reciprocal(recip[:120], sums[:120])

        attO = apool.tile([128, BL, TO], f32, tag="attO")
        nc.gpsimd.tensor_tensor(
            attO[:120], attE[:120],
            recip[:120, :, None].to_broadcast((120, BL, TO)), OP.mult)
        dma_q(att_d[ts(wg, 4)].rearrange("w t b o -> (w t) b o"), attO[:120])

        lacc = spool.tile([128, BL], f32, tag="lacc")
        for ch in range(3):
            pg = pG.tile([128, 400], f32, tag="pG")
            mm(pg, VwT[:, 0, r0:r0 + 128], VoT[:, 0, ts(ch, 400)],
               start=True, stop=False)
            mm(pg, VwT[:, 1, r0:r0 + 128], VoT[:, 1, ts(ch, 400)],
               start=False, stop=True)
            scr = spool.tile([128, 4, TO], f32, tag="scr")
            nc.vector.tensor_tensor(
                scr[:120], attE[:120, ch * 4:(ch + 1) * 4, :],
                pg[:120].rearrange("p (b t) -> p b t", t=TO), OP.mult)
            nc.vector.tensor_reduce(
                lacc[:120, ch * 4:(ch + 1) * 4], scr[:120], axis=AX,
                op=OP.add)
        lout = spool.tile([128, BL], f32, tag="lout")
        nc.vector.tensor_mul(lout[:120], lacc[:120], recip[:120])
        dma_q(lg_d[ts(wg, 4)].rearrange("w t b -> (w t) b"), lout[:120])

        for h in range(2):
            av6 = outpool.tile([128, 6, DV], f32, tag="attV6")
            for tr in range(2):
                pt = ptrf.tile([128, 3, 120], f32, tag="ptrf")
                for i in range(3):
                    b = h * 6 + tr * 3 + i
                    nc.tensor.matmul(pt[:TO, i, :], attE[:120, b, :],
                                     ident[:120, :120], is_transpose=True,
                                     start=(i == 0), stop=(i == 2))
                aT3 = outpool.tile([128, 3, 120], bf16, tag="attT3")
                copy_alt(aT3[:TO], pt[:TO])
                for i in range(3):
                    j = tr * 3 + i
                    b = h * 6 + j
                    pa = pmm.tile([128, 480], f32, tag="pmm")
                    mm(pa[:120, :DV], aT3[:TO, i, :], Vo[:TO, b, :],
                       start=True, stop=True)
                    if b % 2 == 0:
                        nc.vector.tensor_scalar_mul(av6[:120, j, :],
                                                    pa[:120, :DV],
                                                    recip[:120, b:b + 1])
                    else:
                        nc.scalar.activation(av6[:120, j, :], pa[:120, :DV],
                                             AFT.Copy,
                                             scale=recip[:120, b:b + 1])
            dma_q(attV_d[ts(wg, 4), :, h * 6:(h + 1) * 6]
                  .rearrange("w t b d -> (w t) b d"), av6[:120])


def _build():
    from contextlib import ExitStack

    import concourse.mybir as mybir
    import concourse.tile as tile
    from concourse import bacc

    f32 = mybir.dt.float32
    bf16 = mybir.dt.bfloat16
    nc = bacc.Bacc("TRN2", target_bir_lowering=False, debug=False,
                   num_devices=NCORES)

    def di(name, shape, dt):
        return nc.dram_tensor(name, shape, dt, kind="ExternalInput").ap()

    ins = {
        "u16T": di("u16T", [DU, TOK], bf16),
        "o16T": di("o16T", [DO, TOK], bf16),
        "w16T": di("w16T", [DW, WTOK], bf16),
        "Wku16": di("Wku16", [DU, DK], bf16),
        "Wkw16": di("Wkw16", [DW, DK], bf16),
        "Wfo16": di("Wfo16", [DO, DV], bf16),
        "Wfw16": di("Wfw16", [DW, DV], bf16),
        "bku": di("bku", [DK], f32),
        "bkw": di("bkw", [DK], f32),
        "bfo": di("bfo", [DV], f32),
        "bfw": di("bfw", [DV], f32),
    }
    outs = {
        "att": nc.dram_tensor("att", [BW, TW, BL, TO], f32,
                              kind="ExternalOutput").ap(),
        "attV": nc.dram_tensor("attV", [BW, TW, BL, DV], f32,
                               kind="ExternalOutput").ap(),
        "lg": nc.dram_tensor("lg", [BW, TW, BL], f32,
                             kind="ExternalOutput").ap(),
    }

    with tile.TileContext(nc) as tc:
        with ExitStack() as ctx:
            _emit(ctx, tc, outs, ins)
    nc.compile()
    return nc


_NC_CACHE = []
LAST_EXEC_NS = [None]


def _get_nc():
    if not _NC_CACHE:
        _NC_CACHE.append(_build())
    return _NC_CACHE[0]


def _pad_rows(a, rows):
    out = np.zeros((rows,) + a.shape[1:], dtype=a.dtype)
    out[: a.shape[0]] = a
    return out


def kernel(**inputs):
    import os

    from concourse.bass_utils import run_bass_kernel_spmd

    nc = _get_nc()
    bf = ml_dtypes.bfloat16

    u = np.asarray(inputs["u"], np.float32)
    o = np.asarray(inputs["o"], np.float32)
    w = np.asarray(inputs["w"], np.float32)
    mask = np.asarray(inputs["mask"], np.float32)

    shared = {
        "w16T": np.ascontiguousarray(w.reshape(WTOK, DW).T.astype(bf)),
        "Wku16": np.asarray(inputs["Wku"], np.float32).astype(bf),
        "Wkw16": np.asarray(inputs["Wkw"], np.float32).astype(bf),
        "Wfo16": np.asarray(inputs["Wfo"], np.float32).astype(bf),
        "Wfw16": np.asarray(inputs["Wfw"], np.float32).astype(bf),
        "bku": np.ascontiguousarray(np.asarray(inputs["bku"], np.float32)),
        "bkw": np.ascontiguousarray(np.asarray(inputs["bkw"], np.float32)),
        "bfo": np.ascontiguousarray(np.asarray(inputs["bfo"], np.float32)),
        "bfw": np.ascontiguousarray(np.asarray(inputs["bfw"], np.float32)),
    }

    in_maps = []
    for k in range(NCORES):
        m = dict(shared)
        m["u16T"] = np.ascontiguousarray(
            u[k * BL:(k + 1) * BL].reshape(TOK, DU).T.astype(bf))
        m["o16T"] = np.ascontiguousarray(
            o[k * BL:(k + 1) * BL].reshape(TOK, DO).T.astype(bf))
        in_maps.append(m)

    trace = bool(int(os.environ.get("KERNEL_TRACE", "0")))
    res = run_bass_kernel_spmd(nc, in_maps, core_ids=list(range(NCORES)),
                               trace=trace)
    results = res.results
    LAST_EXEC_NS[0] = res.exec_time_ns
    if res.instructions_and_trace is not None:
        print("trace:", res.instructions_and_trace[1])

    att = np.empty((BW, BO, TW, TO), np.float32)
    attV = np.empty((BW, BO, TW, DV), np.float32)
    logits = np.empty((BW, BO, TW), np.float32)
    for k in range(NCORES):
        bs = slice(k * BL, (k + 1) * BL)
        att[:, bs] = results[k]["att"].transpose(0, 2, 1, 3)
        attV[:, bs] = results[k]["attV"].transpose(0, 2, 1, 3)
        logits[:, bs] = results[k]["lg"].transpose(0, 2, 1)

    # contrastive tail on host: log_softmax over Bo, mask, mean of diagonal
    mx = logits.max(axis=1, keepdims=True)
    lse = np.log(np.exp(logits - mx).sum(axis=1, keepdims=True)) + mx
    lsm = logits - lse
    m = mask[:, None, :]
    lsm = (1.0 - m) * lsm
    nn = (1.0 - m).sum(axis=2, keepdims=True)
    lsm = lsm / (nn + 1e-6)
    loss = -np.mean(np.diagonal(lsm.sum(axis=2)))

    return np.float32(loss), att, attV
